# revision 20
# baseline (speedup 1.0000x reference)
"""Trainium2 Bass kernel for residual-VQ autoencoder (nn_Autoencoder_45148696216751).

Pipeline per core (data-parallel over tokens, 8 cores x 2048 tokens, no collectives):
  encoder zT = enc_w @ x.T (+bias)  -> residual rT [d, t] in SBUF
  2x VQ stage:
     score[t,k] = r.c - |c|^2/2 via PE matmuls, codebook streamed from HBM
                  (-|c|^2/2 folded into the stream as a 5th K=128 row)
     mode fp32:  exact fp32 matmuls; per-superchunk max8 + stt index extraction
     mode f32r4: full-speed f32r matmuls; per-superchunk top-8 (max8+max_index),
                 global top-4 by value, exact rescore of the 4 candidates
                 against the exact residual (gathered code rows, gpsimd math)
     indirect-DMA gather of the winning code row per token
     PE transpose of gathered rows; residual update / q_sum accumulation
  decoder out = q_sum @ dec_w.T + dec_b -> DMA out
"""
import sys, types, os

sys.path.insert(0, '/opt/trn_rl_repo')
import numpy as np

import concourse.bass as bass
import concourse.tile as tile
from concourse import bacc, mybir
from concourse.bass_utils import run_bass_kernel_spmd
from concourse.masks import make_identity

f32 = mybir.dt.float32
f32r = mybir.dt.float32r
i32 = mybir.dt.int32
u32 = mybir.dt.uint32
ALU = mybir.AluOpType

NCORES = 8
B, N, D = 4, 4096, 512
T = B * N                 # 16384 tokens
TL = T // NCORES          # 2048 tokens per core
K = 16384                 # codebook size
NT = TL // 128            # 16 token tiles per core
NJ = D // 128             # 4 contraction tiles
SC = 1024                 # superchunk (2 psum banks)
NSC = K // SC             # 16 superchunks
NR = 5                    # codebook stream rows (4 cb + 1 bias)
NUM_Q = 2
NCAND = 4                 # rescued candidates per token (f32r4 mode)

DIST_MODE = os.environ.get("VQ_DIST_MODE", "bf16r4")  # fp32 | f32r4 | bf16r4


def _ensure_axon_hook():
    """Register the NTFF profile hook (missing antenv.axon_hooks shim)."""
    if "antenv.axon_hooks" in sys.modules:
        return
    mod = types.ModuleType("antenv.axon_hooks")
    _h = [None]
    mod.set_axon_ntff_profile_hook = lambda h: _h.__setitem__(0, h)
    mod.get_axon_ntff_profile_hook = lambda: _h[0]
    sys.modules["antenv.axon_hooks"] = mod
    try:
        import antenv
        antenv.axon_hooks = mod
        from trn_agent_boot.trn_boot import _ntff_profile_via_ctypes
        hook = _ntff_profile_via_ctypes('/opt/axon/libaxon_pjrt.so')
        if hook is not None:
            mod.set_axon_ntff_profile_hook(hook)
    except Exception:
        pass


def _build(dist_mode):
    nc = bacc.Bacc("TRN2", target_bir_lowering=False, debug=False,
                   num_devices=NCORES)
    rescue = dist_mode in ("f32r4", "bf16r4")
    md = {"fp32": f32, "f32r4": f32r, "bf16r4": mybir.dt.bfloat16}[dist_mode]

    xT_d = nc.dram_tensor("xT", [128, NJ, TL], f32, kind="ExternalInput")
    cbs_d = nc.dram_tensor("cbs", [128, NR, K], md, kind="ExternalInput")
    cb_d = nc.dram_tensor("cb", [K, D], f32, kind="ExternalInput")
    ewT_d = nc.dram_tensor("ewT", [128, NJ, D], f32, kind="ExternalInput")
    dwT_d = nc.dram_tensor("dwT", [128, NJ, D], f32, kind="ExternalInput")
    eb_d = nc.dram_tensor("eb", [128, NJ], f32, kind="ExternalInput")
    db_d = nc.dram_tensor("db", [128, D], f32, kind="ExternalInput")
    ones_d = nc.dram_tensor("ones", [128, 128], md, kind="ExternalInput")
    if rescue:
        ebf_d = nc.dram_tensor("ebf", [128, D], f32, kind="ExternalInput")
    out_d = nc.dram_tensor("out", [TL, D], f32, kind="ExternalOutput")

    from contextlib import ExitStack
    with tile.TileContext(nc) as tc, ExitStack() as ctx:
        big = ctx.enter_context(tc.tile_pool(name="big", bufs=1))
        scrp = ctx.enter_context(tc.tile_pool(name="scr", bufs=2))
        smallp = ctx.enter_context(tc.tile_pool(name="small", bufs=6))
        qp = ctx.enter_context(tc.tile_pool(name="qp", bufs=3))
        q4p = ctx.enter_context(tc.tile_pool(name="q4p", bufs=2))
        outp = ctx.enter_context(tc.tile_pool(name="outp", bufs=2))
        psc = ctx.enter_context(tc.tile_pool(name="psc", bufs=3, space="PSUM"))
        psm = ctx.enter_context(tc.tile_pool(name="psm", bufs=2, space="PSUM"))

        # ---- persistent tiles
        rT = big.tile([128, NJ, TL], md)      # residual (transposed)
        q1T = big.tile([128, NJ, TL], f32)    # q1T, later q_sumT
        ewT = big.tile([128, NJ, D], f32)
        dwT = big.tile([128, NJ, D], f32)
        eb = big.tile([128, NJ], f32)
        db = big.tile([128, D], f32)
        ones128 = big.tile([128, 128], md)
        ident = big.tile([128, 128], f32)
        m8buf = big.tile([128, NT, NSC, 8], f32)
        if rescue:
            z_td = big.tile([128, NT, D], f32)    # exact residual, [t, d] layout
            ebf = big.tile([128, D], f32)
            idxbuf = big.tile([128, NT, NSC, 8], mybir.dt.uint16)
            iota_off8 = big.tile([128, NSC, 8], f32)
            wi32 = big.tile([128, NT], i32)
        else:
            iota_sc = big.tile([128, SC], f32)
            iota_off = big.tile([128, NSC], f32)
            sidxbuf = big.tile([128, NT, NSC], f32)
            idx32 = big.tile([128, NT], i32)

        nc.sync.dma_start(ewT[:], ewT_d.ap())
        nc.sync.dma_start(dwT[:], dwT_d.ap())
        nc.sync.dma_start(eb[:], eb_d.ap())
        nc.sync.dma_start(db[:], db_d.ap())
        nc.sync.dma_start(ones128[:], ones_d.ap())
        if rescue:
            nc.sync.dma_start(ebf[:], ebf_d.ap())
        make_identity(nc, ident[:])
        if rescue:
            nc.gpsimd.iota(iota_off8[:], pattern=[[SC, NSC], [0, 8]], base=0,
                           channel_multiplier=0,
                           allow_small_or_imprecise_dtypes=True)
        else:
            nc.gpsimd.iota(iota_sc[:], pattern=[[1, SC]], base=0,
                           channel_multiplier=0,
                           allow_small_or_imprecise_dtypes=True)
            nc.gpsimd.iota(iota_off[:], pattern=[[SC, NSC]], base=0,
                           channel_multiplier=0,
                           allow_small_or_imprecise_dtypes=True)

        # ---- encoder(s), xT freed before the codebook stream pool opens
        HL = TL // 2
        with tc.tile_pool(name="xp", bufs=1) as xp:
            for half in range(2):
                xth = xp.tile([128, NJ, HL], f32, tag="x")
                nc.sync.dma_start(xth[:],
                                  xT_d.ap()[:, :, half * HL:(half + 1) * HL])
                # zT[d', t] = enc_w @ x.T, bias per-partition, -> rT
                for i in range(NJ):
                    for c in range(HL // 512):
                        co = half * HL + c * 512
                        psz = psm.tile([128, 512], f32, tag="m")
                        for j in range(NJ):
                            nc.tensor.matmul(
                                psz[:], lhsT=ewT[:, j, i * 128:(i + 1) * 128],
                                rhs=xth[:, j, c * 512:(c + 1) * 512],
                                start=(j == 0), stop=(j == NJ - 1))
                        if rescue:
                            nc.vector.tensor_scalar(
                                out=rT[:, i, co:co + 512], in0=psz[:],
                                scalar1=eb[:, i:i + 1], scalar2=None,
                                op0=ALU.add)
                        else:
                            nc.scalar.add(rT[:, i, co:co + 512], psz[:],
                                          eb[:, i:i + 1])
                if rescue:
                    # z in [t, d] layout for exact rescoring
                    for t2 in range(HL // 128):
                        t = half * (HL // 128) + t2
                        psz = psm.tile([128, 512], f32, tag="m")
                        for j in range(NJ):
                            nc.tensor.matmul(
                                psz[:], lhsT=xth[:, j, t2 * 128:(t2 + 1) * 128],
                                rhs=ewT[:, j, :],
                                start=(j == 0), stop=(j == NJ - 1))
                        nc.vector.tensor_add(z_td[:, t, :], psz[:], ebf[:])

        with tc.tile_pool(name="cbp", bufs=6) as cbp:
            # ---- VQ stages
            for s in range(NUM_Q):
              for tg in range(2):
                tset = range(tg * NT // 2, (tg + 1) * NT // 2)
                # score sweep
                for sc in range(NSC):
                    cbts = []
                    for h in range(SC // 512):
                        cbt = cbp.tile([128, NR, 512], md, tag="cbt")
                        ko = sc * SC + h * 512
                        nc.sync.dma_start(cbt[:],
                                          cbs_d.ap()[:, :, ko:ko + 512])
                        cbts.append(cbt)
                    for t in tset:
                        ps = psc.tile([128, SC], f32, tag="sc")
                        for h in range(SC // 512):
                            pslice = ps[:, h * 512:(h + 1) * 512]
                            cbt = cbts[h]
                            for j in range(NJ):
                                nc.tensor.matmul(
                                    pslice,
                                    lhsT=rT[:, j, t * 128:(t + 1) * 128],
                                    rhs=cbt[:, j, :],
                                    start=(j == 0), stop=False)
                            nc.tensor.matmul(
                                pslice, lhsT=ones128[:],
                                rhs=cbt[:, NR - 1, :],
                                start=False, stop=True)
                        nc.vector.max(out=m8buf[:, t, sc, :], in_=ps[:])
                        if rescue:
                            nc.vector.max_index(out=idxbuf[:, t, sc, :],
                                                in_max=m8buf[:, t, sc, :],
                                                in_values=ps[:])
                        else:
                            scr = scrp.tile([128, SC], f32, tag="s")
                            nc.vector.scalar_tensor_tensor(
                                out=scr[:], in0=ps[:],
                                scalar=m8buf[:, t, sc, 0:1], in1=iota_sc[:],
                                op0=ALU.is_ge, op1=ALU.mult,
                                accum_out=sidxbuf[:, t, sc:sc + 1])

                # combine + gather (+rescore) + transpose per token tile
                for t in tset:
                    if rescue:
                        idxf = smallp.tile([128, NSC, 8], f32, tag="idxf")
                        nc.gpsimd.tensor_copy(idxf[:], idxbuf[:, t, :, :])
                        nc.gpsimd.tensor_add(idxf[:], idxf[:], iota_off8[:])
                        g8 = smallp.tile([128, 8], f32, tag="g8")
                        nc.vector.max(out=g8[:], in_=m8buf[:, t, :, :])
                        cs = smallp.tile([128, NCAND], f32, tag="cs")
                        junk = scrp.tile([128, NSC, 8], f32, tag="s")
                        for k_ in range(NCAND):
                            nc.vector.scalar_tensor_tensor(
                                out=junk[:], in0=m8buf[:, t, :, :],
                                scalar=g8[:, k_:k_ + 1], in1=idxf[:],
                                op0=ALU.is_ge, op1=ALU.mult,
                                accum_out=cs[:, k_:k_ + 1])
                        idx4f = smallp.tile([128, NCAND], f32, tag="i4f")
                        nc.vector.tensor_copy(idx4f[:, 0:1], cs[:, 0:1])
                        nc.vector.tensor_sub(idx4f[:, 1:NCAND], cs[:, 1:NCAND],
                                             cs[:, 0:NCAND - 1])
                        idx4 = smallp.tile([128, NCAND], i32, tag="i4")
                        nc.vector.tensor_copy(idx4[:], idx4f[:])
                        q4 = q4p.tile([128, NCAND, D], f32, tag="q4")
                        for k_ in range(NCAND):
                            nc.gpsimd.indirect_dma_start(
                                out=q4[:, k_, :], out_offset=None,
                                in_=cb_d.ap(),
                                in_offset=bass.IndirectOffsetOnAxis(
                                    ap=idx4[:, k_:k_ + 1], axis=0))
                        # exact rescore: score_k = sum_d q*r - 0.5*sum_d q^2
                        u4 = scrp.tile([128, NCAND, D], f32, tag="u4")
                        s1 = smallp.tile([128, NCAND], f32, tag="s1")
                        s2 = smallp.tile([128, NCAND], f32, tag="s2")
                        for k_ in range(NCAND):
                            nc.gpsimd.tensor_mul(u4[:, k_, :], q4[:, k_, :],
                                                 z_td[:, t, :])
                            nc.scalar.activation(
                                out=u4[:, k_, :], in_=u4[:, k_, :],
                                func=mybir.ActivationFunctionType.Copy,
                                accum_out=s1[:, k_:k_ + 1])
                            nc.scalar.activation(
                                out=u4[:, k_, :], in_=q4[:, k_, :],
                                func=mybir.ActivationFunctionType.Square,
                                accum_out=s2[:, k_:k_ + 1])
                        sc4 = smallp.tile([128, NCAND], f32, tag="sc4")
                        nc.vector.scalar_tensor_tensor(
                            out=sc4[:], in0=s2[:], scalar=-0.5, in1=s1[:],
                            op0=ALU.mult, op1=ALU.add)
                        gm = smallp.tile([128, 1], f32, tag="gm")
                        nc.vector.tensor_reduce(gm[:], sc4[:],
                                                axis=mybir.AxisListType.X,
                                                op=ALU.max)
                        junk4 = smallp.tile([128, NCAND], f32, tag="j4")
                        widxf = smallp.tile([128, 1], f32, tag="wf")
                        nc.vector.scalar_tensor_tensor(
                            out=junk4[:], in0=sc4[:], scalar=gm[:],
                            in1=idx4f[:], op0=ALU.is_ge, op1=ALU.mult,
                            accum_out=widxf[:])
                        nc.vector.tensor_copy(wi32[:, t:t + 1], widxf[:])
                        idx_ap = wi32[:, t:t + 1]
                    else:
                        mv = smallp.tile([128, NSC], f32, tag="mv")
                        nc.vector.tensor_copy(mv[:], m8buf[:, t, :, 0:1])
                        sg = smallp.tile([128, NSC], f32, tag="sg")
                        nc.vector.tensor_add(sg[:], sidxbuf[:, t, :],
                                             iota_off[:])
                        g8 = smallp.tile([128, 8], f32, tag="g8")
                        nc.vector.max(out=g8[:], in_=mv[:])
                        tmp = smallp.tile([128, NSC], f32, tag="tmp")
                        idxf = smallp.tile([128, 1], f32, tag="ix")
                        nc.vector.scalar_tensor_tensor(
                            out=tmp[:], in0=mv[:], scalar=g8[:, 0:1],
                            in1=sg[:], op0=ALU.is_ge, op1=ALU.mult,
                            accum_out=idxf[:])
                        nc.vector.tensor_copy(idx32[:, t:t + 1], idxf[:])
                        idx_ap = idx32[:, t:t + 1]

                    q_t = qp.tile([128, D], f32, tag="qt")
                    nc.gpsimd.indirect_dma_start(
                        out=q_t[:], out_offset=None, in_=cb_d.ap(),
                        in_offset=bass.IndirectOffsetOnAxis(ap=idx_ap, axis=0))
                    if rescue and s == 0:
                        # exact residual for stage-2 rescoring
                        nc.gpsimd.tensor_sub(z_td[:, t, :], z_td[:, t, :],
                                             q_t[:])
                    for j in range(NJ):
                        tp = psm.tile([128, 512], f32, tag="m")
                        nc.tensor.transpose(tp[:, 0:128],
                                            q_t[:, j * 128:(j + 1) * 128],
                                            ident[:])
                        tsl = slice(t * 128, (t + 1) * 128)
                        if s == 0:
                            nc.scalar.copy(q1T[:, j, tsl], tp[:, 0:128])
                            nc.vector.tensor_sub(rT[:, j, tsl], rT[:, j, tsl],
                                                 tp[:, 0:128])
                        else:
                            nc.vector.tensor_add(q1T[:, j, tsl],
                                                 q1T[:, j, tsl], tp[:, 0:128])

        # ---- decoder: out[t, d'] = q_sumT.T @ dec_w.T + dec_b
        for t in range(NT):
            pso = psm.tile([128, 512], f32, tag="m")
            for j in range(NJ):
                nc.tensor.matmul(pso[:], lhsT=q1T[:, j, t * 128:(t + 1) * 128],
                                 rhs=dwT[:, j, :],
                                 start=(j == 0), stop=(j == NJ - 1))
            o_t = outp.tile([128, D], f32, tag="o")
            nc.vector.tensor_add(o_t[:], pso[:], db[:])
            nc.sync.dma_start(out_d.ap()[t * 128:(t + 1) * 128, :], o_t[:])

    nc.compile()
    return nc


_CACHE = {}


def _get_nc():
    key = DIST_MODE
    if key not in _CACHE:
        _ensure_axon_hook()
        _CACHE[key] = _build(DIST_MODE)
    return _CACHE[key]


def _trunc_mant(a, bits):
    """Truncate fp32 mantissa to `bits` explicit bits (round to nearest)."""
    u = a.astype(np.float32).view(np.uint32).copy()
    shift = np.uint32(23 - bits)
    rb = np.uint32(1 << (23 - bits - 1))
    u = ((u + rb) >> shift) << shift
    return u.view(np.float32)


def _host_prep(x, enc_w, enc_b, codebook, dec_w, dec_b):
    x = np.asarray(x, np.float32)
    enc_w = np.asarray(enc_w, np.float32)
    enc_b = np.asarray(enc_b, np.float32)
    cb = np.ascontiguousarray(np.asarray(codebook, np.float32))
    dec_w = np.asarray(dec_w, np.float32)
    dec_b = np.asarray(dec_b, np.float32)
    rescue = DIST_MODE in ("f32r4", "bf16r4")

    flat = x.reshape(T, D)
    csq = (cb.astype(np.float64) ** 2).sum(-1).astype(np.float32)
    bias = -0.5 * csq

    cbT = np.ascontiguousarray(cb.T)                      # [D, K]
    cbs = np.zeros((128, NR, K), np.float32)
    cbs[:, :NJ, :] = cbT.reshape(NJ, 128, K).transpose(1, 0, 2)
    if DIST_MODE == "f32r4":
        # split the bias exactly across two f32r-representable rows
        bh = _trunc_mant(bias, 10)
        cbs[0, NJ, :] = bh
        cbs[1, NJ, :] = bias - bh
    elif DIST_MODE == "bf16r4":
        import ml_dtypes
        bh = np.asarray(bias, ml_dtypes.bfloat16).astype(np.float32)
        cbs[0, NJ, :] = bh
        cbs[1, NJ, :] = bias - bh
        cbs = np.asarray(cbs, ml_dtypes.bfloat16)
    else:
        cbs[0, NJ, :] = bias

    ewT = np.ascontiguousarray(
        enc_w.T.reshape(NJ, 128, D).transpose(1, 0, 2))   # [128, NJ, D]
    dwT = np.ascontiguousarray(
        dec_w.T.reshape(NJ, 128, D).transpose(1, 0, 2))
    eb = np.ascontiguousarray(enc_b.reshape(NJ, 128).T)   # [128, NJ]
    db = np.ascontiguousarray(np.broadcast_to(dec_b, (128, D)))
    ones = np.ones((128, 128), np.float32)
    if DIST_MODE == "bf16r4":
        import ml_dtypes
        ones = np.asarray(ones, ml_dtypes.bfloat16)

    common = {"cbs": cbs, "cb": cb, "ewT": ewT, "dwT": dwT,
              "eb": eb, "db": db, "ones": ones}
    if rescue:
        common["ebf"] = np.ascontiguousarray(np.broadcast_to(enc_b, (128, D)))

    in_maps = []
    for s in range(NCORES):
        shard = flat[s * TL:(s + 1) * TL]                 # [TL, D]
        xT = np.ascontiguousarray(
            shard.T.reshape(NJ, 128, TL).transpose(1, 0, 2))
        in_maps.append({"xT": xT, **common})
    return in_maps


def _run(inputs, trace=False):
    nc = _get_nc()
    in_maps = _host_prep(**inputs)
    res = run_bass_kernel_spmd(nc, in_maps, list(range(NCORES)), trace=trace)
    outs = [res.results[s]["out"] for s in range(NCORES)]
    full = np.concatenate(outs, axis=0).reshape(B, N, D)
    return full, res


def kernel(**inputs) -> np.ndarray:
    out, _ = _run(inputs, trace=False)
    return out


def kernel_traced(**inputs):
    out, res = _run(inputs, trace=True)
    return out, res


# revision 21
# speedup vs baseline: 1.0125x; 1.0125x over previous
"""Trainium2 Bass kernel for residual-VQ autoencoder (nn_Autoencoder_45148696216751).

Pipeline per core (data-parallel over tokens, 8 cores x 2048 tokens, no collectives):
  encoder zT = enc_w @ x.T (+bias)  -> residual rT [d, t] in SBUF
  2x VQ stage:
     score[t,k] = r.c - |c|^2/2 via PE matmuls, codebook streamed from HBM
                  (-|c|^2/2 folded into the stream as a 5th K=128 row)
     mode fp32:  exact fp32 matmuls; per-superchunk max8 + stt index extraction
     mode f32r4: full-speed f32r matmuls; per-superchunk top-8 (max8+max_index),
                 global top-4 by value, exact rescore of the 4 candidates
                 against the exact residual (gathered code rows, gpsimd math)
     indirect-DMA gather of the winning code row per token
     PE transpose of gathered rows; residual update / q_sum accumulation
  decoder out = q_sum @ dec_w.T + dec_b -> DMA out
"""
import sys, types, os

sys.path.insert(0, '/opt/trn_rl_repo')
import numpy as np

import concourse.bass as bass
import concourse.tile as tile
from concourse import bacc, mybir
from concourse.bass_utils import run_bass_kernel_spmd
from concourse.masks import make_identity

f32 = mybir.dt.float32
f32r = mybir.dt.float32r
i32 = mybir.dt.int32
u32 = mybir.dt.uint32
ALU = mybir.AluOpType

NCORES = 8
B, N, D = 4, 4096, 512
T = B * N                 # 16384 tokens
TL = T // NCORES          # 2048 tokens per core
K = 16384                 # codebook size
NT = TL // 128            # 16 token tiles per core
NJ = D // 128             # 4 contraction tiles
SC = 1024                 # superchunk (2 psum banks)
NSC = K // SC             # 16 superchunks
NR = 5                    # codebook stream rows (4 cb + 1 bias)
NUM_Q = 2
NCAND = 4                 # rescued candidates per token (f32r4 mode)

DIST_MODE = os.environ.get("VQ_DIST_MODE", "bf16r4")  # fp32 | f32r4 | bf16r4


def _ensure_axon_hook():
    """Register the NTFF profile hook (missing antenv.axon_hooks shim)."""
    if "antenv.axon_hooks" in sys.modules:
        return
    mod = types.ModuleType("antenv.axon_hooks")
    _h = [None]
    mod.set_axon_ntff_profile_hook = lambda h: _h.__setitem__(0, h)
    mod.get_axon_ntff_profile_hook = lambda: _h[0]
    sys.modules["antenv.axon_hooks"] = mod
    try:
        import antenv
        antenv.axon_hooks = mod
        from trn_agent_boot.trn_boot import _ntff_profile_via_ctypes
        hook = _ntff_profile_via_ctypes('/opt/axon/libaxon_pjrt.so')
        if hook is not None:
            mod.set_axon_ntff_profile_hook(hook)
    except Exception:
        pass


def _build(dist_mode):
    nc = bacc.Bacc("TRN2", target_bir_lowering=False, debug=False,
                   num_devices=NCORES)
    rescue = dist_mode in ("f32r4", "bf16r4")
    md = {"fp32": f32, "f32r4": f32r, "bf16r4": mybir.dt.bfloat16}[dist_mode]

    xT_d = nc.dram_tensor("xT", [128, NJ, TL], f32, kind="ExternalInput")
    cbs_d = nc.dram_tensor("cbs", [128, NR, K], md, kind="ExternalInput")
    cb_d = nc.dram_tensor("cb", [K, D], f32, kind="ExternalInput")
    ewT_d = nc.dram_tensor("ewT", [128, NJ, D], f32, kind="ExternalInput")
    dwT_d = nc.dram_tensor("dwT", [128, NJ, D], f32, kind="ExternalInput")
    eb_d = nc.dram_tensor("eb", [128, NJ], f32, kind="ExternalInput")
    db_d = nc.dram_tensor("db", [128, D], f32, kind="ExternalInput")
    ones_d = nc.dram_tensor("ones", [128, 128], md, kind="ExternalInput")
    if rescue:
        ebf_d = nc.dram_tensor("ebf", [128, D], f32, kind="ExternalInput")
    out_d = nc.dram_tensor("out", [TL, D], f32, kind="ExternalOutput")

    from contextlib import ExitStack
    with tile.TileContext(nc) as tc, ExitStack() as ctx:
        big = ctx.enter_context(tc.tile_pool(name="big", bufs=1))
        scrp = ctx.enter_context(tc.tile_pool(name="scr", bufs=2))
        smallp = ctx.enter_context(tc.tile_pool(name="small", bufs=8))
        qp = ctx.enter_context(tc.tile_pool(name="qp", bufs=3))
        q4p = ctx.enter_context(tc.tile_pool(name="q4p", bufs=3))
        outp = ctx.enter_context(tc.tile_pool(name="outp", bufs=2))
        psc = ctx.enter_context(tc.tile_pool(name="psc", bufs=3, space="PSUM"))
        psm = ctx.enter_context(tc.tile_pool(name="psm", bufs=2, space="PSUM"))

        # ---- persistent tiles
        rT = big.tile([128, NJ, TL], md)      # residual (transposed)
        q1T = big.tile([128, NJ, TL], f32)    # q1T, later q_sumT
        ewT = big.tile([128, NJ, D], f32)
        dwT = big.tile([128, NJ, D], f32)
        eb = big.tile([128, NJ], f32)
        db = big.tile([128, D], f32)
        ones128 = big.tile([128, 128], md)
        ident = big.tile([128, 128], f32)
        m8buf = big.tile([128, NT, NSC, 8], f32)
        if rescue:
            z_td = big.tile([128, NT, D], f32)    # exact residual, [t, d] layout
            ebf = big.tile([128, D], f32)
            idxbuf = big.tile([128, NT, NSC, 8], mybir.dt.uint16)
            iota_off8 = big.tile([128, NSC, 8], f32)
            wi32 = big.tile([128, NT], i32)
        else:
            iota_sc = big.tile([128, SC], f32)
            iota_off = big.tile([128, NSC], f32)
            sidxbuf = big.tile([128, NT, NSC], f32)
            idx32 = big.tile([128, NT], i32)

        nc.sync.dma_start(ewT[:], ewT_d.ap())
        nc.sync.dma_start(dwT[:], dwT_d.ap())
        nc.sync.dma_start(eb[:], eb_d.ap())
        nc.sync.dma_start(db[:], db_d.ap())
        nc.sync.dma_start(ones128[:], ones_d.ap())
        if rescue:
            nc.sync.dma_start(ebf[:], ebf_d.ap())
        make_identity(nc, ident[:])
        if rescue:
            nc.gpsimd.iota(iota_off8[:], pattern=[[SC, NSC], [0, 8]], base=0,
                           channel_multiplier=0,
                           allow_small_or_imprecise_dtypes=True)
        else:
            nc.gpsimd.iota(iota_sc[:], pattern=[[1, SC]], base=0,
                           channel_multiplier=0,
                           allow_small_or_imprecise_dtypes=True)
            nc.gpsimd.iota(iota_off[:], pattern=[[SC, NSC]], base=0,
                           channel_multiplier=0,
                           allow_small_or_imprecise_dtypes=True)

        # ---- encoder(s), xT freed before the codebook stream pool opens
        HL = TL // 2
        with tc.tile_pool(name="xp", bufs=1) as xp:
            for half in range(2):
                xth = xp.tile([128, NJ, HL], f32, tag="x")
                nc.sync.dma_start(xth[:],
                                  xT_d.ap()[:, :, half * HL:(half + 1) * HL])
                # zT[d', t] = enc_w @ x.T, bias per-partition, -> rT
                # column-outer: completes early token tiles' residual first
                for c in range(HL // 512):
                    for i in range(NJ):
                        co = half * HL + c * 512
                        psz = psm.tile([128, 512], f32, tag="m")
                        for j in range(NJ):
                            nc.tensor.matmul(
                                psz[:], lhsT=ewT[:, j, i * 128:(i + 1) * 128],
                                rhs=xth[:, j, c * 512:(c + 1) * 512],
                                start=(j == 0), stop=(j == NJ - 1))
                        if rescue:
                            nc.vector.tensor_scalar(
                                out=rT[:, i, co:co + 512], in0=psz[:],
                                scalar1=eb[:, i:i + 1], scalar2=None,
                                op0=ALU.add)
                        else:
                            nc.scalar.add(rT[:, i, co:co + 512], psz[:],
                                          eb[:, i:i + 1])
                if rescue:
                    # z in [t, d] layout for exact rescoring
                    for t2 in range(HL // 128):
                        t = half * (HL // 128) + t2
                        psz = psm.tile([128, 512], f32, tag="m")
                        for j in range(NJ):
                            nc.tensor.matmul(
                                psz[:], lhsT=xth[:, j, t2 * 128:(t2 + 1) * 128],
                                rhs=ewT[:, j, :],
                                start=(j == 0), stop=(j == NJ - 1))
                        nc.vector.tensor_add(z_td[:, t, :], psz[:], ebf[:])

        with tc.tile_pool(name="cbp", bufs=6) as cbp:
            # ---- VQ stages
            for s in range(NUM_Q):
              for tg in range(2):
                tset = range(tg * NT // 2, (tg + 1) * NT // 2)
                # score sweep
                for sc in range(NSC):
                    cbts = []
                    for h in range(SC // 512):
                        cbt = cbp.tile([128, NR, 512], md, tag="cbt")
                        ko = sc * SC + h * 512
                        nc.sync.dma_start(cbt[:],
                                          cbs_d.ap()[:, :, ko:ko + 512])
                        cbts.append(cbt)
                    for t in tset:
                        ps = psc.tile([128, SC], f32, tag="sc")
                        for h in range(SC // 512):
                            pslice = ps[:, h * 512:(h + 1) * 512]
                            cbt = cbts[h]
                            for j in range(NJ):
                                nc.tensor.matmul(
                                    pslice,
                                    lhsT=rT[:, j, t * 128:(t + 1) * 128],
                                    rhs=cbt[:, j, :],
                                    start=(j == 0), stop=False)
                            nc.tensor.matmul(
                                pslice, lhsT=ones128[:],
                                rhs=cbt[:, NR - 1, :],
                                start=False, stop=True)
                        nc.vector.max(out=m8buf[:, t, sc, :], in_=ps[:])
                        if rescue:
                            nc.vector.max_index(out=idxbuf[:, t, sc, :],
                                                in_max=m8buf[:, t, sc, :],
                                                in_values=ps[:])
                        else:
                            scr = scrp.tile([128, SC], f32, tag="s")
                            nc.vector.scalar_tensor_tensor(
                                out=scr[:], in0=ps[:],
                                scalar=m8buf[:, t, sc, 0:1], in1=iota_sc[:],
                                op0=ALU.is_ge, op1=ALU.mult,
                                accum_out=sidxbuf[:, t, sc:sc + 1])

                # combine + gather (+rescore) + transpose per token tile
                for t in tset:
                    if rescue:
                        idxf = smallp.tile([128, NSC, 8], f32, tag="idxf")
                        nc.gpsimd.tensor_copy(idxf[:], idxbuf[:, t, :, :])
                        nc.gpsimd.tensor_add(idxf[:], idxf[:], iota_off8[:])
                        g8 = smallp.tile([128, 8], f32, tag="g8")
                        nc.vector.max(out=g8[:], in_=m8buf[:, t, :, :])
                        cs = smallp.tile([128, NCAND], f32, tag="cs")
                        junk = scrp.tile([128, NSC, 8], f32, tag="s")
                        for k_ in range(NCAND):
                            nc.vector.scalar_tensor_tensor(
                                out=junk[:], in0=m8buf[:, t, :, :],
                                scalar=g8[:, k_:k_ + 1], in1=idxf[:],
                                op0=ALU.is_ge, op1=ALU.mult,
                                accum_out=cs[:, k_:k_ + 1])
                        idx4f = smallp.tile([128, NCAND], f32, tag="i4f")
                        nc.vector.tensor_copy(idx4f[:, 0:1], cs[:, 0:1])
                        nc.vector.tensor_sub(idx4f[:, 1:NCAND], cs[:, 1:NCAND],
                                             cs[:, 0:NCAND - 1])
                        idx4 = smallp.tile([128, NCAND], i32, tag="i4")
                        nc.vector.tensor_copy(idx4[:], idx4f[:])
                        q4 = q4p.tile([128, NCAND, D], f32, tag="q4")
                        for k_ in range(NCAND):
                            nc.gpsimd.indirect_dma_start(
                                out=q4[:, k_, :], out_offset=None,
                                in_=cb_d.ap(),
                                in_offset=bass.IndirectOffsetOnAxis(
                                    ap=idx4[:, k_:k_ + 1], axis=0))
                        # exact rescore: score_k = sum_d q*r - 0.5*sum_d q^2
                        u4 = scrp.tile([128, NCAND, D], f32, tag="u4")
                        s1 = smallp.tile([128, NCAND], f32, tag="s1")
                        s2 = smallp.tile([128, NCAND], f32, tag="s2")
                        for k_ in range(NCAND):
                            nc.gpsimd.tensor_mul(u4[:, k_, :], q4[:, k_, :],
                                                 z_td[:, t, :])
                            nc.scalar.activation(
                                out=u4[:, k_, :], in_=u4[:, k_, :],
                                func=mybir.ActivationFunctionType.Copy,
                                accum_out=s1[:, k_:k_ + 1])
                            nc.scalar.activation(
                                out=u4[:, k_, :], in_=q4[:, k_, :],
                                func=mybir.ActivationFunctionType.Square,
                                accum_out=s2[:, k_:k_ + 1])
                        sc4 = smallp.tile([128, NCAND], f32, tag="sc4")
                        nc.vector.scalar_tensor_tensor(
                            out=sc4[:], in0=s2[:], scalar=-0.5, in1=s1[:],
                            op0=ALU.mult, op1=ALU.add)
                        gm = smallp.tile([128, 1], f32, tag="gm")
                        nc.vector.tensor_reduce(gm[:], sc4[:],
                                                axis=mybir.AxisListType.X,
                                                op=ALU.max)
                        junk4 = smallp.tile([128, NCAND], f32, tag="j4")
                        widxf = smallp.tile([128, 1], f32, tag="wf")
                        nc.vector.scalar_tensor_tensor(
                            out=junk4[:], in0=sc4[:], scalar=gm[:],
                            in1=idx4f[:], op0=ALU.is_ge, op1=ALU.mult,
                            accum_out=widxf[:])
                        nc.vector.tensor_copy(wi32[:, t:t + 1], widxf[:])
                        idx_ap = wi32[:, t:t + 1]
                    else:
                        mv = smallp.tile([128, NSC], f32, tag="mv")
                        nc.vector.tensor_copy(mv[:], m8buf[:, t, :, 0:1])
                        sg = smallp.tile([128, NSC], f32, tag="sg")
                        nc.vector.tensor_add(sg[:], sidxbuf[:, t, :],
                                             iota_off[:])
                        g8 = smallp.tile([128, 8], f32, tag="g8")
                        nc.vector.max(out=g8[:], in_=mv[:])
                        tmp = smallp.tile([128, NSC], f32, tag="tmp")
                        idxf = smallp.tile([128, 1], f32, tag="ix")
                        nc.vector.scalar_tensor_tensor(
                            out=tmp[:], in0=mv[:], scalar=g8[:, 0:1],
                            in1=sg[:], op0=ALU.is_ge, op1=ALU.mult,
                            accum_out=idxf[:])
                        nc.vector.tensor_copy(idx32[:, t:t + 1], idxf[:])
                        idx_ap = idx32[:, t:t + 1]

                    q_t = qp.tile([128, D], f32, tag="qt")
                    nc.gpsimd.indirect_dma_start(
                        out=q_t[:], out_offset=None, in_=cb_d.ap(),
                        in_offset=bass.IndirectOffsetOnAxis(ap=idx_ap, axis=0))
                    if rescue and s == 0:
                        # exact residual for stage-2 rescoring
                        nc.vector.tensor_sub(z_td[:, t, :], z_td[:, t, :],
                                             q_t[:])
                    for j in range(NJ):
                        tp = psm.tile([128, 512], f32, tag="m")
                        nc.tensor.transpose(tp[:, 0:128],
                                            q_t[:, j * 128:(j + 1) * 128],
                                            ident[:])
                        tsl = slice(t * 128, (t + 1) * 128)
                        if s == 0:
                            nc.scalar.copy(q1T[:, j, tsl], tp[:, 0:128])
                            nc.vector.tensor_sub(rT[:, j, tsl], rT[:, j, tsl],
                                                 tp[:, 0:128])
                        else:
                            nc.vector.tensor_add(q1T[:, j, tsl],
                                                 q1T[:, j, tsl], tp[:, 0:128])

        # ---- decoder: out[t, d'] = q_sumT.T @ dec_w.T + dec_b
        for t in range(NT):
            pso = psm.tile([128, 512], f32, tag="m")
            for j in range(NJ):
                nc.tensor.matmul(pso[:], lhsT=q1T[:, j, t * 128:(t + 1) * 128],
                                 rhs=dwT[:, j, :],
                                 start=(j == 0), stop=(j == NJ - 1))
            o_t = outp.tile([128, D], f32, tag="o")
            nc.vector.tensor_add(o_t[:], pso[:], db[:])
            nc.sync.dma_start(out_d.ap()[t * 128:(t + 1) * 128, :], o_t[:])

    nc.compile()
    return nc


_CACHE = {}


def _get_nc():
    key = DIST_MODE
    if key not in _CACHE:
        _ensure_axon_hook()
        _CACHE[key] = _build(DIST_MODE)
    return _CACHE[key]


def _trunc_mant(a, bits):
    """Truncate fp32 mantissa to `bits` explicit bits (round to nearest)."""
    u = a.astype(np.float32).view(np.uint32).copy()
    shift = np.uint32(23 - bits)
    rb = np.uint32(1 << (23 - bits - 1))
    u = ((u + rb) >> shift) << shift
    return u.view(np.float32)


def _host_prep(x, enc_w, enc_b, codebook, dec_w, dec_b):
    x = np.asarray(x, np.float32)
    enc_w = np.asarray(enc_w, np.float32)
    enc_b = np.asarray(enc_b, np.float32)
    cb = np.ascontiguousarray(np.asarray(codebook, np.float32))
    dec_w = np.asarray(dec_w, np.float32)
    dec_b = np.asarray(dec_b, np.float32)
    rescue = DIST_MODE in ("f32r4", "bf16r4")

    flat = x.reshape(T, D)
    csq = (cb.astype(np.float64) ** 2).sum(-1).astype(np.float32)
    bias = -0.5 * csq

    cbT = np.ascontiguousarray(cb.T)                      # [D, K]
    cbs = np.zeros((128, NR, K), np.float32)
    cbs[:, :NJ, :] = cbT.reshape(NJ, 128, K).transpose(1, 0, 2)
    if DIST_MODE == "f32r4":
        # split the bias exactly across two f32r-representable rows
        bh = _trunc_mant(bias, 10)
        cbs[0, NJ, :] = bh
        cbs[1, NJ, :] = bias - bh
    elif DIST_MODE == "bf16r4":
        import ml_dtypes
        bh = np.asarray(bias, ml_dtypes.bfloat16).astype(np.float32)
        cbs[0, NJ, :] = bh
        cbs[1, NJ, :] = bias - bh
        cbs = np.asarray(cbs, ml_dtypes.bfloat16)
    else:
        cbs[0, NJ, :] = bias

    ewT = np.ascontiguousarray(
        enc_w.T.reshape(NJ, 128, D).transpose(1, 0, 2))   # [128, NJ, D]
    dwT = np.ascontiguousarray(
        dec_w.T.reshape(NJ, 128, D).transpose(1, 0, 2))
    eb = np.ascontiguousarray(enc_b.reshape(NJ, 128).T)   # [128, NJ]
    db = np.ascontiguousarray(np.broadcast_to(dec_b, (128, D)))
    ones = np.ones((128, 128), np.float32)
    if DIST_MODE == "bf16r4":
        import ml_dtypes
        ones = np.asarray(ones, ml_dtypes.bfloat16)

    common = {"cbs": cbs, "cb": cb, "ewT": ewT, "dwT": dwT,
              "eb": eb, "db": db, "ones": ones}
    if rescue:
        common["ebf"] = np.ascontiguousarray(np.broadcast_to(enc_b, (128, D)))

    in_maps = []
    for s in range(NCORES):
        shard = flat[s * TL:(s + 1) * TL]                 # [TL, D]
        xT = np.ascontiguousarray(
            shard.T.reshape(NJ, 128, TL).transpose(1, 0, 2))
        in_maps.append({"xT": xT, **common})
    return in_maps


def _run(inputs, trace=False):
    nc = _get_nc()
    in_maps = _host_prep(**inputs)
    res = run_bass_kernel_spmd(nc, in_maps, list(range(NCORES)), trace=trace)
    outs = [res.results[s]["out"] for s in range(NCORES)]
    full = np.concatenate(outs, axis=0).reshape(B, N, D)
    return full, res


def kernel(**inputs) -> np.ndarray:
    out, _ = _run(inputs, trace=False)
    return out


def kernel_traced(**inputs):
    out, res = _run(inputs, trace=True)
    return out, res


# revision 22
# speedup vs baseline: 1.0183x; 1.0057x over previous
"""Trainium2 Bass kernel for residual-VQ autoencoder (nn_Autoencoder_45148696216751).

Pipeline per core (data-parallel over tokens, 8 cores x 2048 tokens, no collectives):
  encoder zT = enc_w @ x.T (+bias)  -> residual rT [d, t] in SBUF
  2x VQ stage:
     score[t,k] = r.c - |c|^2/2 via PE matmuls, codebook streamed from HBM
                  (-|c|^2/2 folded into the stream as a 5th K=128 row)
     mode fp32:  exact fp32 matmuls; per-superchunk max8 + stt index extraction
     mode f32r4: full-speed f32r matmuls; per-superchunk top-8 (max8+max_index),
                 global top-4 by value, exact rescore of the 4 candidates
                 against the exact residual (gathered code rows, gpsimd math)
     indirect-DMA gather of the winning code row per token
     PE transpose of gathered rows; residual update / q_sum accumulation
  decoder out = q_sum @ dec_w.T + dec_b -> DMA out
"""
import sys, types, os

sys.path.insert(0, '/opt/trn_rl_repo')
import numpy as np

import concourse.bass as bass
import concourse.tile as tile
from concourse import bacc, mybir
from concourse.bass_utils import run_bass_kernel_spmd
from concourse.masks import make_identity

f32 = mybir.dt.float32
f32r = mybir.dt.float32r
i32 = mybir.dt.int32
u32 = mybir.dt.uint32
ALU = mybir.AluOpType

NCORES = 8
B, N, D = 4, 4096, 512
T = B * N                 # 16384 tokens
TL = T // NCORES          # 2048 tokens per core
K = 16384                 # codebook size
NT = TL // 128            # 16 token tiles per core
NJ = D // 128             # 4 contraction tiles
SC = 1024                 # superchunk (2 psum banks)
NSC = K // SC             # 16 superchunks
NR = 5                    # codebook stream rows (4 cb + 1 bias)
NUM_Q = 2
NCAND = 4                 # rescued candidates per token (f32r4 mode)

DIST_MODE = os.environ.get("VQ_DIST_MODE", "bf16r4")  # fp32 | f32r4 | bf16r4


def _ensure_axon_hook():
    """Register the NTFF profile hook (missing antenv.axon_hooks shim)."""
    if "antenv.axon_hooks" in sys.modules:
        return
    mod = types.ModuleType("antenv.axon_hooks")
    _h = [None]
    mod.set_axon_ntff_profile_hook = lambda h: _h.__setitem__(0, h)
    mod.get_axon_ntff_profile_hook = lambda: _h[0]
    sys.modules["antenv.axon_hooks"] = mod
    try:
        import antenv
        antenv.axon_hooks = mod
        from trn_agent_boot.trn_boot import _ntff_profile_via_ctypes
        hook = _ntff_profile_via_ctypes('/opt/axon/libaxon_pjrt.so')
        if hook is not None:
            mod.set_axon_ntff_profile_hook(hook)
    except Exception:
        pass


def _build(dist_mode):
    nc = bacc.Bacc("TRN2", target_bir_lowering=False, debug=False,
                   num_devices=NCORES)
    rescue = dist_mode in ("f32r4", "bf16r4")
    md = {"fp32": f32, "f32r4": f32r, "bf16r4": mybir.dt.bfloat16}[dist_mode]

    xT_d = nc.dram_tensor("xT", [128, NJ, TL], f32, kind="ExternalInput")
    cbs_d = nc.dram_tensor("cbs", [128, NR, K], md, kind="ExternalInput")
    cb_d = nc.dram_tensor("cb", [K, D], f32, kind="ExternalInput")
    ewT_d = nc.dram_tensor("ewT", [128, NJ, D], f32, kind="ExternalInput")
    dwT_d = nc.dram_tensor("dwT", [128, NJ, D], f32, kind="ExternalInput")
    eb_d = nc.dram_tensor("eb", [128, NJ], f32, kind="ExternalInput")
    db_d = nc.dram_tensor("db", [128, D], f32, kind="ExternalInput")
    ones_d = nc.dram_tensor("ones", [128, 128], md, kind="ExternalInput")
    if rescue:
        ebf_d = nc.dram_tensor("ebf", [128, D], f32, kind="ExternalInput")
    out_d = nc.dram_tensor("out", [TL, D], f32, kind="ExternalOutput")

    from contextlib import ExitStack
    with tile.TileContext(nc) as tc, ExitStack() as ctx:
        big = ctx.enter_context(tc.tile_pool(name="big", bufs=1))
        scrp = ctx.enter_context(tc.tile_pool(name="scr", bufs=2))
        smallp = ctx.enter_context(tc.tile_pool(name="small", bufs=8))
        qp = ctx.enter_context(tc.tile_pool(name="qp", bufs=3))
        q4p = ctx.enter_context(tc.tile_pool(name="q4p", bufs=3))
        outp = ctx.enter_context(tc.tile_pool(name="outp", bufs=2))
        psc = ctx.enter_context(tc.tile_pool(name="psc", bufs=3, space="PSUM"))
        psm = ctx.enter_context(tc.tile_pool(name="psm", bufs=2, space="PSUM"))

        # ---- persistent tiles
        rT = big.tile([128, NJ, TL], md)      # residual (transposed)
        q1T = big.tile([128, NJ, TL], f32)    # q1T, later q_sumT
        ewT = big.tile([128, NJ, D], f32)
        dwT = big.tile([128, NJ, D], f32)
        eb = big.tile([128, NJ], f32)
        db = big.tile([128, D], f32)
        ones128 = big.tile([128, 128], md)
        ident = big.tile([128, 128], f32)
        m8buf = big.tile([128, NT, NSC, 8], f32)
        if rescue:
            z_td = big.tile([128, NT, D], f32)    # exact residual, [t, d] layout
            ebf = big.tile([128, D], f32)
            idxbuf = big.tile([128, NT, NSC, 8], mybir.dt.uint16)
            iota_off8 = big.tile([128, NSC, 8], f32)
            wi32 = big.tile([128, NT], i32)
        else:
            iota_sc = big.tile([128, SC], f32)
            iota_off = big.tile([128, NSC], f32)
            sidxbuf = big.tile([128, NT, NSC], f32)
            idx32 = big.tile([128, NT], i32)

        nc.sync.dma_start(ewT[:], ewT_d.ap())
        nc.sync.dma_start(dwT[:], dwT_d.ap())
        nc.sync.dma_start(eb[:], eb_d.ap())
        nc.sync.dma_start(db[:], db_d.ap())
        nc.sync.dma_start(ones128[:], ones_d.ap())
        if rescue:
            nc.sync.dma_start(ebf[:], ebf_d.ap())
        make_identity(nc, ident[:])
        if rescue:
            nc.gpsimd.iota(iota_off8[:], pattern=[[SC, NSC], [0, 8]], base=0,
                           channel_multiplier=0,
                           allow_small_or_imprecise_dtypes=True)
        else:
            nc.gpsimd.iota(iota_sc[:], pattern=[[1, SC]], base=0,
                           channel_multiplier=0,
                           allow_small_or_imprecise_dtypes=True)
            nc.gpsimd.iota(iota_off[:], pattern=[[SC, NSC]], base=0,
                           channel_multiplier=0,
                           allow_small_or_imprecise_dtypes=True)

        # ---- encoder(s), xT freed before the codebook stream pool opens
        HL = TL // 2
        with tc.tile_pool(name="xp", bufs=1) as xp:
            for half in range(2):
                xth = xp.tile([128, NJ, HL], f32, tag="x")
                nc.sync.dma_start(xth[:],
                                  xT_d.ap()[:, :, half * HL:(half + 1) * HL])
                # zT[d', t] = enc_w @ x.T, bias per-partition, -> rT
                # column-outer: completes early token tiles' residual first
                for c in range(HL // 512):
                    for i in range(NJ):
                        co = half * HL + c * 512
                        psz = psm.tile([128, 512], f32, tag="m")
                        for j in range(NJ):
                            nc.tensor.matmul(
                                psz[:], lhsT=ewT[:, j, i * 128:(i + 1) * 128],
                                rhs=xth[:, j, c * 512:(c + 1) * 512],
                                start=(j == 0), stop=(j == NJ - 1))
                        if rescue:
                            nc.vector.tensor_scalar(
                                out=rT[:, i, co:co + 512], in0=psz[:],
                                scalar1=eb[:, i:i + 1], scalar2=None,
                                op0=ALU.add)
                        else:
                            nc.scalar.add(rT[:, i, co:co + 512], psz[:],
                                          eb[:, i:i + 1])
                if rescue:
                    # z in [t, d] layout for exact rescoring
                    for t2 in range(HL // 128):
                        t = half * (HL // 128) + t2
                        psz = psm.tile([128, 512], f32, tag="m")
                        for j in range(NJ):
                            nc.tensor.matmul(
                                psz[:], lhsT=xth[:, j, t2 * 128:(t2 + 1) * 128],
                                rhs=ewT[:, j, :],
                                start=(j == 0), stop=(j == NJ - 1))
                        nc.vector.tensor_add(z_td[:, t, :], psz[:], ebf[:])

        with tc.tile_pool(name="cbp", bufs=6) as cbp:
            # ---- VQ stages
            for s in range(NUM_Q):
              for tg in range(2):
                tset = range(tg * NT // 2, (tg + 1) * NT // 2)
                # score sweep
                for sc in range(NSC):
                    cbts = []
                    for h in range(SC // 512):
                        cbt = cbp.tile([128, NR, 512], md, tag="cbt")
                        ko = sc * SC + h * 512
                        nc.sync.dma_start(cbt[:],
                                          cbs_d.ap()[:, :, ko:ko + 512])
                        cbts.append(cbt)
                    for t in tset:
                        ps = psc.tile([128, SC], f32, tag="sc")
                        for h in range(SC // 512):
                            pslice = ps[:, h * 512:(h + 1) * 512]
                            cbt = cbts[h]
                            for j in range(NJ):
                                nc.tensor.matmul(
                                    pslice,
                                    lhsT=rT[:, j, t * 128:(t + 1) * 128],
                                    rhs=cbt[:, j, :],
                                    start=(j == 0), stop=False)
                            nc.tensor.matmul(
                                pslice, lhsT=ones128[:],
                                rhs=cbt[:, NR - 1, :],
                                start=False, stop=True)
                        nc.vector.max(out=m8buf[:, t, sc, :], in_=ps[:])
                        if rescue:
                            nc.vector.max_index(out=idxbuf[:, t, sc, :],
                                                in_max=m8buf[:, t, sc, :],
                                                in_values=ps[:])
                        else:
                            scr = scrp.tile([128, SC], f32, tag="s")
                            nc.vector.scalar_tensor_tensor(
                                out=scr[:], in0=ps[:],
                                scalar=m8buf[:, t, sc, 0:1], in1=iota_sc[:],
                                op0=ALU.is_ge, op1=ALU.mult,
                                accum_out=sidxbuf[:, t, sc:sc + 1])

                # combine + gather (+rescore) + transpose per token tile
                for t in tset:
                    if rescue:
                        idxf = smallp.tile([128, NSC, 8], f32, tag="idxf")
                        nc.gpsimd.tensor_copy(idxf[:], idxbuf[:, t, :, :])
                        nc.gpsimd.tensor_add(idxf[:], idxf[:], iota_off8[:])
                        g8 = smallp.tile([128, 8], f32, tag="g8")
                        nc.vector.max(out=g8[:], in_=m8buf[:, t, :, :])
                        cs = smallp.tile([128, NCAND], f32, tag="cs")
                        junk = scrp.tile([128, NSC, 8], f32, tag="s")
                        for k_ in range(NCAND):
                            nc.vector.scalar_tensor_tensor(
                                out=junk[:], in0=m8buf[:, t, :, :],
                                scalar=g8[:, k_:k_ + 1], in1=idxf[:],
                                op0=ALU.is_ge, op1=ALU.mult,
                                accum_out=cs[:, k_:k_ + 1])
                        idx4f = smallp.tile([128, NCAND], f32, tag="i4f")
                        nc.vector.tensor_copy(idx4f[:, 0:1], cs[:, 0:1])
                        nc.vector.tensor_sub(idx4f[:, 1:NCAND], cs[:, 1:NCAND],
                                             cs[:, 0:NCAND - 1])
                        idx4 = smallp.tile([128, NCAND], i32, tag="i4")
                        nc.vector.tensor_copy(idx4[:], idx4f[:])
                        q4 = q4p.tile([128, NCAND, D], f32, tag="q4")
                        for k_ in range(NCAND):
                            nc.gpsimd.indirect_dma_start(
                                out=q4[:, k_, :], out_offset=None,
                                in_=cb_d.ap(),
                                in_offset=bass.IndirectOffsetOnAxis(
                                    ap=idx4[:, k_:k_ + 1], axis=0))
                        # exact rescore: score_k = sum_d q*r - 0.5*sum_d q^2
                        u4 = scrp.tile([128, NCAND, D], f32, tag="u4")
                        s1 = smallp.tile([128, NCAND], f32, tag="s1")
                        s2 = smallp.tile([128, NCAND], f32, tag="s2")
                        for k_ in range(NCAND):
                            nc.gpsimd.tensor_mul(u4[:, k_, :], q4[:, k_, :],
                                                 z_td[:, t, :])
                            nc.scalar.activation(
                                out=u4[:, k_, :], in_=u4[:, k_, :],
                                func=mybir.ActivationFunctionType.Copy,
                                accum_out=s1[:, k_:k_ + 1])
                            nc.scalar.activation(
                                out=u4[:, k_, :], in_=q4[:, k_, :],
                                func=mybir.ActivationFunctionType.Square,
                                accum_out=s2[:, k_:k_ + 1])
                        sc4 = smallp.tile([128, NCAND], f32, tag="sc4")
                        nc.vector.scalar_tensor_tensor(
                            out=sc4[:], in0=s2[:], scalar=-0.5, in1=s1[:],
                            op0=ALU.mult, op1=ALU.add)
                        gm = smallp.tile([128, 1], f32, tag="gm")
                        nc.vector.tensor_reduce(gm[:], sc4[:],
                                                axis=mybir.AxisListType.X,
                                                op=ALU.max)
                        junk4 = smallp.tile([128, NCAND], f32, tag="j4")
                        widxf = smallp.tile([128, 1], f32, tag="wf")
                        nc.vector.scalar_tensor_tensor(
                            out=junk4[:], in0=sc4[:], scalar=gm[:],
                            in1=idx4f[:], op0=ALU.is_ge, op1=ALU.mult,
                            accum_out=widxf[:])
                        nc.vector.tensor_copy(wi32[:, t:t + 1], widxf[:])
                        idx_ap = wi32[:, t:t + 1]
                    else:
                        mv = smallp.tile([128, NSC], f32, tag="mv")
                        nc.vector.tensor_copy(mv[:], m8buf[:, t, :, 0:1])
                        sg = smallp.tile([128, NSC], f32, tag="sg")
                        nc.vector.tensor_add(sg[:], sidxbuf[:, t, :],
                                             iota_off[:])
                        g8 = smallp.tile([128, 8], f32, tag="g8")
                        nc.vector.max(out=g8[:], in_=mv[:])
                        tmp = smallp.tile([128, NSC], f32, tag="tmp")
                        idxf = smallp.tile([128, 1], f32, tag="ix")
                        nc.vector.scalar_tensor_tensor(
                            out=tmp[:], in0=mv[:], scalar=g8[:, 0:1],
                            in1=sg[:], op0=ALU.is_ge, op1=ALU.mult,
                            accum_out=idxf[:])
                        nc.vector.tensor_copy(idx32[:, t:t + 1], idxf[:])
                        idx_ap = idx32[:, t:t + 1]

                    q_t = qp.tile([128, D], f32, tag="qt")
                    nc.gpsimd.indirect_dma_start(
                        out=q_t[:], out_offset=None, in_=cb_d.ap(),
                        in_offset=bass.IndirectOffsetOnAxis(ap=idx_ap, axis=0))
                    if rescue and s == 0:
                        # exact residual for stage-2 rescoring
                        nc.vector.tensor_sub(z_td[:, t, :], z_td[:, t, :],
                                             q_t[:])
                    tp4 = psm.tile([128, NJ, 128], f32, tag="m")
                    for j in range(NJ):
                        nc.tensor.transpose(tp4[:, j, :],
                                            q_t[:, j * 128:(j + 1) * 128],
                                            ident[:])
                    tsl = slice(t * 128, (t + 1) * 128)
                    if s == 0:
                        nc.scalar.copy(q1T[:, :, tsl], tp4[:])
                        nc.vector.tensor_sub(rT[:, :, tsl], rT[:, :, tsl],
                                             tp4[:])
                    else:
                        nc.vector.tensor_add(q1T[:, :, tsl],
                                             q1T[:, :, tsl], tp4[:])

        # ---- decoder: out[t, d'] = q_sumT.T @ dec_w.T + dec_b
        for t in range(NT):
            pso = psm.tile([128, 512], f32, tag="m")
            for j in range(NJ):
                nc.tensor.matmul(pso[:], lhsT=q1T[:, j, t * 128:(t + 1) * 128],
                                 rhs=dwT[:, j, :],
                                 start=(j == 0), stop=(j == NJ - 1))
            o_t = outp.tile([128, D], f32, tag="o")
            nc.vector.tensor_add(o_t[:], pso[:], db[:])
            nc.sync.dma_start(out_d.ap()[t * 128:(t + 1) * 128, :], o_t[:])

    nc.compile()
    return nc


_CACHE = {}


def _get_nc():
    key = DIST_MODE
    if key not in _CACHE:
        _ensure_axon_hook()
        _CACHE[key] = _build(DIST_MODE)
    return _CACHE[key]


def _trunc_mant(a, bits):
    """Truncate fp32 mantissa to `bits` explicit bits (round to nearest)."""
    u = a.astype(np.float32).view(np.uint32).copy()
    shift = np.uint32(23 - bits)
    rb = np.uint32(1 << (23 - bits - 1))
    u = ((u + rb) >> shift) << shift
    return u.view(np.float32)


def _host_prep(x, enc_w, enc_b, codebook, dec_w, dec_b):
    x = np.asarray(x, np.float32)
    enc_w = np.asarray(enc_w, np.float32)
    enc_b = np.asarray(enc_b, np.float32)
    cb = np.ascontiguousarray(np.asarray(codebook, np.float32))
    dec_w = np.asarray(dec_w, np.float32)
    dec_b = np.asarray(dec_b, np.float32)
    rescue = DIST_MODE in ("f32r4", "bf16r4")

    flat = x.reshape(T, D)
    csq = (cb.astype(np.float64) ** 2).sum(-1).astype(np.float32)
    bias = -0.5 * csq

    cbT = np.ascontiguousarray(cb.T)                      # [D, K]
    cbs = np.zeros((128, NR, K), np.float32)
    cbs[:, :NJ, :] = cbT.reshape(NJ, 128, K).transpose(1, 0, 2)
    if DIST_MODE == "f32r4":
        # split the bias exactly across two f32r-representable rows
        bh = _trunc_mant(bias, 10)
        cbs[0, NJ, :] = bh
        cbs[1, NJ, :] = bias - bh
    elif DIST_MODE == "bf16r4":
        import ml_dtypes
        bh = np.asarray(bias, ml_dtypes.bfloat16).astype(np.float32)
        cbs[0, NJ, :] = bh
        cbs[1, NJ, :] = bias - bh
        cbs = np.asarray(cbs, ml_dtypes.bfloat16)
    else:
        cbs[0, NJ, :] = bias

    ewT = np.ascontiguousarray(
        enc_w.T.reshape(NJ, 128, D).transpose(1, 0, 2))   # [128, NJ, D]
    dwT = np.ascontiguousarray(
        dec_w.T.reshape(NJ, 128, D).transpose(1, 0, 2))
    eb = np.ascontiguousarray(enc_b.reshape(NJ, 128).T)   # [128, NJ]
    db = np.ascontiguousarray(np.broadcast_to(dec_b, (128, D)))
    ones = np.ones((128, 128), np.float32)
    if DIST_MODE == "bf16r4":
        import ml_dtypes
        ones = np.asarray(ones, ml_dtypes.bfloat16)

    common = {"cbs": cbs, "cb": cb, "ewT": ewT, "dwT": dwT,
              "eb": eb, "db": db, "ones": ones}
    if rescue:
        common["ebf"] = np.ascontiguousarray(np.broadcast_to(enc_b, (128, D)))

    in_maps = []
    for s in range(NCORES):
        shard = flat[s * TL:(s + 1) * TL]                 # [TL, D]
        xT = np.ascontiguousarray(
            shard.T.reshape(NJ, 128, TL).transpose(1, 0, 2))
        in_maps.append({"xT": xT, **common})
    return in_maps


def _run(inputs, trace=False):
    nc = _get_nc()
    in_maps = _host_prep(**inputs)
    res = run_bass_kernel_spmd(nc, in_maps, list(range(NCORES)), trace=trace)
    outs = [res.results[s]["out"] for s in range(NCORES)]
    full = np.concatenate(outs, axis=0).reshape(B, N, D)
    return full, res


def kernel(**inputs) -> np.ndarray:
    out, _ = _run(inputs, trace=False)
    return out


def kernel_traced(**inputs):
    out, res = _run(inputs, trace=True)
    return out, res
